# revision 16
# baseline (speedup 1.0000x reference)
"""DenseGTVConv Trainium2 kernel (v8, band-symmetric + ReduceScatter).

out = (I - (D - A~)) @ (x @ W) + bias,  A~ = adj / clamp(pairwise_L1(xW), 1e-3)

D is symmetric: each unordered pair {i,j} is computed ONCE.  Core c (of 4
per batch) owns rows [256c, 256c+256) and the pairs (i, j) with
(j-i) mod 1024 in [1, 511] (plus the d==512 tie when i < 512).  Inputs are
rotated by r0 so each core sees a contiguous 768-column band; adj and adjT
slices are host-masked to the owned band, so unwanted D entries multiply
to zero.  Per owned pair the core computes BOTH mod[i,j] (direct, via adj)
and mod[j,i] (transposed, via adjT); direct contributions use the usual
transpose+matmul, transposed contributions contract naturally over i
(no transpose).  Per-core partials [1024, 65] (out | deg) are scattered to
absolute rows via indirect DMA (host-provided indices), ReduceScattered
across the 4-core group, and the final combine runs on-chip.

Hot-loop structure (relu-identity + sliding-E PSUM reduction, fp8
DoubleRow duos, S1/S2 folds, software pipelining) follows v5-v7, with a
640-column window per 128-row block (PSUM split 512 + 128).

Self-contained: hardcoded shapes for B=2, N=1024, F_in=128, F_out=64.
"""
import sys

sys.path.insert(0, "/opt/trn_rl_repo")

from contextlib import ExitStack

import numpy as np
import ml_dtypes

import concourse.bass as bass
import concourse.bacc as bacc
import concourse.tile as tile
from concourse.masks import make_identity
from concourse import mybir
from concourse._compat import with_exitstack
from concourse.bass_utils import run_bass_kernel_spmd

F32 = mybir.dt.float32
BF16 = mybir.dt.bfloat16
FP8 = mybir.dt.float8e4
U32 = mybir.dt.uint32

B, N, C, F = 2, 1024, 128, 64
R = 256
NPAIR = R // 2
EPS = 1e-3

W = 768  # band columns per core (rotated)
WIN = 640  # window per 128-row q-block: jj in [128q, 128q+640)
K0, K1 = 512, 128  # PSUM column split of the window
NCHD = 5  # direct 128-col chunks per q (640/128)
NCHT = 6  # total jj chunks per core (768/128)
FD = F + 1  # 65: out | deg

# xallb layout [128, W + R + 2F = 1152]:
#   0:768 xT' (rotated), 768:1024 xrT, 1024:1088 W, 1088:1152 bias(row0)
XALL2 = W + R + 2 * F

ND8 = 10  # fp8 duos per q (pairs d, d+32) on ACT
NVB = 64 - 2 * ND8  # bf16 vector pairs per q

NWARM = 4
KSPLIT = 24
V_PRE = 14
PE_PRE1 = 6
PE_PRE2 = 7


def _expand_sched():
    slots = [dict(kind="a8", d=d) for d in range(ND8)]
    used = set(range(ND8)) | set(range(32, 32 + ND8))
    slots += [dict(kind="vb", t=t) for t in range(64) if t not in used]
    return slots


def _pe_order(slots):
    vb = [s for s in slots if s["kind"] == "vb"]
    a8 = [s for s in slots if s["kind"] == "a8"]
    if not a8 or not vb:
        return slots
    keyed = [((i + 0.5) / len(vb), s) for i, s in enumerate(vb)]
    keyed += [((j + 1.5) / (len(a8) + 1), s) for j, s in enumerate(a8)]
    return [s for _, s in sorted(keyed, key=lambda p: p[0])]


def _act_recip(sc, out, in_, bias, scale=1.0):
    inputs = [sc.lower_ap(in_)]
    for arg in (bias, scale, 0.0):
        inputs.append(mybir.ImmediateValue(dtype=mybir.dt.float32, value=arg))
    return sc.add_instruction(
        mybir.InstActivation(
            name=sc.bass.get_next_instruction_name(),
            func=mybir.ActivationFunctionType.Reciprocal,
            ins=inputs,
            outs=[sc.lower_ap(out)],
        )
    )


@with_exitstack
def _body(ctx: ExitStack, tc: "tile.TileContext", io: dict):
    nc = tc.nc
    const = ctx.enter_context(tc.tile_pool(name="const", bufs=1))
    tmpv_pool = ctx.enter_context(tc.tile_pool(name="tmpv", bufs=14))
    tmp8a_pool = ctx.enter_context(tc.tile_pool(name="tmp8a", bufs=5))
    recip_pool = ctx.enter_context(tc.tile_pool(name="recip", bufs=2))
    modbf_pool = ctx.enter_context(tc.tile_pool(name="modbf", bufs=2))
    modt_pool = ctx.enter_context(tc.tile_pool(name="modt", bufs=3))
    setup_ps = ctx.enter_context(tc.tile_pool(name="sps", bufs=2, space="PSUM"))
    ad_ps = ctx.enter_context(tc.tile_pool(name="adps", bufs=2, space="PSUM"))
    trfin_ps = ctx.enter_context(tc.tile_pool(name="trfin", bufs=2, space="PSUM"))
    finT_ps = ctx.enter_context(tc.tile_pool(name="ftps", bufs=1, space="PSUM"))

    xallb = const.tile([128, XALL2], BF16)
    adjq = [const.tile([128, W], BF16, tag=f"adj{q}", name=f"adj{q}") for q in range(2)]
    adjTq = [
        const.tile([128, W], BF16, tag=f"adjT{q}", name=f"adjT{q}") for q in range(2)
    ]
    junk = const.tile([128, 512], BF16)
    nc.vector.memset(junk[:], 0.0)

    # ---- input DMAs ----
    nc.sync.dma_start(xallb[:, W:XALL2], io["xallb"][:, W:XALL2])
    nc.sync.dma_start(xallb[:, 0:256], io["xallb"][:, 0:256])
    nc.scalar.dma_start(xallb[:, 256:W], io["xallb"][:, 256:W])
    nc.scalar.dma_start(adjq[0][:], io["adjb"][0:128, :])
    nc.scalar.dma_start(adjq[1][:], io["adjb"][128:256, :])
    nc.gpsimd.dma_start(adjTq[0][:], io["adjTb"][0:128, :])
    nc.gpsimd.dma_start(adjTq[1][:], io["adjTb"][128:256, :])

    # ---- PE warmup ----
    for w in range(NWARM):
        ps = ad_ps.tile([128, 512], F32, tag="adps0", name=f"warm{w}")
        nc.tensor.matmul(ps[:], junk[:, 0:128], junk[:], start=True, stop=True)

    # ---- constants ----
    onesrow = const.tile([1, 512], BF16)
    nc.vector.memset(onesrow[:], 1.0)
    neghalf = const.tile([1, 128], BF16)
    nc.vector.memset(neghalf[:], -0.5)
    ones64b = const.tile([64, 1], BF16)
    nc.vector.memset(ones64b[:], 1.0)
    ones64f = const.tile([64, 1], F32)
    nc.vector.memset(ones64f[:], 1.0)
    s1row = const.tile([1, W], BF16)
    s2row = const.tile([1, R], BF16)

    identb = const.tile([128, 128], BF16)
    make_identity(nc, identb[:])
    Eb = const.tile([128, 254], BF16)
    nc.gpsimd.memset(Eb[:], 0.0)
    nc.gpsimd.memset(Eb[0:64, 126:127], 1.0)
    nc.gpsimd.memset(Eb[64:128, 127:128], 1.0)
    Eb8 = const.tile([128, 320], FP8)
    nc.gpsimd.memset(Eb8[:], 0.0)
    nc.gpsimd.memset(Eb8[0:64, 62:63], 1.0)
    nc.gpsimd.memset(Eb8[64:128, 63:64], 1.0)
    nc.gpsimd.memset(Eb8[0:64, 254:255], 1.0)
    nc.gpsimd.memset(Eb8[64:128, 255:256], 1.0)

    xTb = xallb[:, 0:W]
    xrTb = xallb[:, W : W + R]
    w_sb = xallb[:, W + R : W + R + F]
    bias_sb = xallb[0:1, W + R + F : W + R + 2 * F]

    # ---- dbl' [128, 768] ----
    w2 = const.tile([128, 128], BF16)
    nc.vector.tensor_copy(w2[:, 0:F], w_sb)
    nc.vector.tensor_copy(w2[:, F : 2 * F], w_sb)
    dbl = const.tile([128, W], BF16)
    ps = ad_ps.tile([128, 512], F32, tag="adps0", name="dblps")
    nc.tensor.matmul(ps[:], w2[:], xTb[:, 0:512], start=True, stop=True)
    nc.vector.tensor_copy(dbl[:, 0:512], ps[:])
    ps = ad_ps.tile([128, 512], F32, tag="adps0", name="dblps")
    nc.tensor.matmul(ps[:, 0:256], w2[:], xTb[:, 512:W], start=True, stop=True)
    nc.vector.tensor_copy(dbl[:, 512:W], ps[:, 0:256])

    # ---- xwT_rows / S / negS (exact i-side) ----
    xwT_rows = const.tile([64, R], F32)
    ps = ad_ps.tile([128, 512], F32, tag="adps0", name="xwTps")
    nc.tensor.matmul(ps[0:64, 0:R], w_sb, xrTb[:], start=True, stop=True)
    nc.vector.tensor_copy(xwT_rows[:], ps[0:64, 0:R])
    S_bf = const.tile([128, NPAIR], F32)
    nc.vector.tensor_copy(S_bf[0:64, :], xwT_rows[:, 0:R:2])
    nc.vector.tensor_copy(S_bf[64:128, :], xwT_rows[:, 1:R:2])
    negS = const.tile([128, NPAIR], F32)
    nc.vector.tensor_scalar(negS[:], S_bf[:], -1.0, None, mybir.AluOpType.mult)

    # ---- deferred setup: s1/s2 ----
    s1ps = []

    def emit_s1s2_mms():
        ps = setup_ps.tile([128, 512], F32, tag="sps", name="s1psA")
        nc.tensor.matmul(ps[0:1, :], ones64b[:], dbl[0:64, 0:512], start=True, stop=True)
        s1ps.append(ps)
        ps = setup_ps.tile([128, 512], F32, tag="sps", name="s1psB")
        nc.tensor.matmul(
            ps[0:1, 0:256], ones64b[:], dbl[0:64, 512:W], start=True, stop=True
        )
        s1ps.append(ps)
        ps = setup_ps.tile([128, 512], F32, tag="sps", name="s2ps")
        nc.tensor.matmul(ps[0:1, 0:R], ones64f[:], xwT_rows[:], start=True, stop=True)
        s1ps.append(ps)

    def emit_s1s2_copies():
        nc.scalar.copy(s1row[:, 0:512], s1ps[0][0:1, :])
        nc.scalar.copy(s1row[:, 512:W], s1ps[1][0:1, 0:256])
        nc.scalar.activation(
            s2row[:], s1ps[2][0:1, 0:R],
            mybir.ActivationFunctionType.Copy, bias=0.5 * EPS, scale=0.5,
        )

    # ---- deferred setup: xwb1 (6 chunks), xw_rows, bias, xwr1 ----
    xwb1 = const.tile([128, NCHT * FD], BF16)
    xw_rows = const.tile([128, 2 * F], F32)
    negxw = const.tile([128, 2 * F], F32)
    xwb_pre = const.tile([128, 2 * F], F32)
    xwr1 = const.tile([128, 2 * FD], BF16)
    ones1 = const.tile([1, 128], BF16)
    nc.scalar.activation(
        ones1[:], xallb[0:1, 0:128], mybir.ActivationFunctionType.Copy,
        bias=1.0, scale=0.0,
    )
    xwps = []

    def emit_xw_mms():
        ps_xw = setup_ps.tile([128, 512], F32, tag="sps", name="psxw")
        for c in range(NCHT):
            nc.tensor.matmul(
                ps_xw[:, F * c : F * c + F], xTb[:, 128 * c : 128 * c + 128], w_sb,
                start=True, stop=True,
            )
        ps2 = setup_ps.tile([128, 512], F32, tag="sps", name="psxr")
        for q in range(2):
            nc.tensor.matmul(
                ps2[:, F * q : F * q + F], xrTb[:, 128 * q : 128 * q + 128], w_sb,
                start=True, stop=True,
            )
            nc.tensor.matmul(
                ps2[:, 2 * F + F * q : 3 * F + F * q], ones1[:], bias_sb,
                start=True, stop=True,
            )
        xwps.extend([ps_xw, ps2])

    def emit_xw_copies_scalar():
        nc.scalar.copy(
            xwb1[:].rearrange("p (c f) -> p c f", c=NCHT)[:, :, 0:F],
            xwps[0][:, 0 : NCHT * F].rearrange("p (c f) -> p c f", c=NCHT),
        )

    def emit_xw_copies_v():
        nc.vector.memset(xwb1[:, F : NCHT * FD : FD], 1.0)
        nc.vector.tensor_copy(xw_rows[:], xwps[1][:, 0 : 2 * F])
        nc.vector.tensor_scalar(negxw[:], xw_rows[:], -1.0, None, mybir.AluOpType.mult)
        nc.vector.tensor_tensor(
            xwb_pre[:], xw_rows[:], xwps[1][:, 2 * F : 4 * F], mybir.AluOpType.add
        )
        for q in range(2):
            nc.vector.tensor_copy(
                xwr1[:, FD * q : FD * q + F], xw_rows[:, F * q : F * q + F]
            )
        nc.vector.memset(xwr1[:, F : 2 * FD : FD], 1.0)

    out_sb = [const.tile([128, F], F32, tag=f"osb{q}", name=f"osb{q}") for q in range(2)]

    # ================= hot loop =================
    slots = [_expand_sched() for _ in range(2)]
    orders = [_pe_order(slots[q]) for q in range(2)]
    adps = {}
    for q in range(2):
        adps[q] = [
            ad_ps.tile(
                [128, 512] if k == 0 else [128, 128], F32,
                tag=f"adps{k}", name=f"adps{q}_{k}",
                bufs=(2 if k == 0 else 1),
            )
            for k in range(2)
        ]

    def emit_v_producers(q, idxs):
        for j, s in enumerate(x for x in slots[q] if x["kind"] == "vb"):
            if j not in idxs:
                continue
            t = 64 * q + s["t"]
            tmpb = tmpv_pool.tile([128, WIN], BF16, tag="tv", name="tv")
            nc.vector.tensor_scalar(
                tmpb[:], dbl[:, 128 * q : 128 * q + WIN], S_bf[:, t : t + 1], 0.0,
                mybir.AluOpType.subtract, mybir.AluOpType.max,
            )
            s["tile"] = tmpb

    def emit_a_producers(q, idxs):
        for j, s in enumerate(x for x in slots[q] if x["kind"] == "a8"):
            if j not in idxs:
                continue
            d = s["d"]
            duo = tmp8a_pool.tile([128, 2 * WIN], FP8, tag="ta", name="ta")
            for half, t in ((0, 64 * q + d), (1, 64 * q + d + 32)):
                nc.scalar.activation(
                    duo[:, WIN * half : WIN * half + WIN],
                    dbl[:, 128 * q : 128 * q + WIN],
                    mybir.ActivationFunctionType.Relu,
                    bias=negS[:, t : t + 1], scale=1.0,
                )
            s["tile"] = duo

    KR = {0: (0, K0), 1: (K0, WIN)}  # window column ranges per psum half

    def emit_pe_weave(q, lo, hi, ks=(0, 1)):
        order = orders[q]
        for i in range(lo, hi):
            s = order[i]
            for k in ks:
                a, b = KR[k]
                if s["kind"] == "vb":
                    r = s["t"]
                    nc.tensor.matmul(
                        adps[q][k][:, 0 : b - a],
                        Eb[:, 126 - 2 * r : 254 - 2 * r],
                        s["tile"][:, a:b],
                        start=(i == 0),
                        stop=False,
                    )
                else:
                    d = s["d"]
                    lhsT = Eb8[:, 62 - 2 * d : 62 - 2 * d + 256].rearrange(
                        "p (s m) -> p s m", s=2
                    )
                    rhs = s["tile"][:].rearrange("p (s n) -> p s n", s=2)[:, :, a:b]
                    nc.tensor.matmul(
                        adps[q][k][:, 0 : b - a], lhsT, rhs,
                        start=(i == 0), stop=False,
                        perf_mode=mybir.MatmulPerfMode.DoubleRow,
                    )

    def emit_pe_folds(q, k):
        a, b = KR[k]
        nc.tensor.matmul(
            adps[q][k][:, 0 : b - a], neghalf[:],
            s1row[0:1, 128 * q + a : 128 * q + b],
            start=False, stop=False,
        )
        nc.tensor.matmul(
            adps[q][k][:, 0 : b - a], s2row[0:1, 128 * q : 128 * q + 128],
            onesrow[0:1, 0 : b - a],
            start=False, stop=True,
        )

    recips = {}
    modbfs = {}
    modtrs = {}

    def emit_recip(q, k, on_v=False):
        # adj is pre-scaled by 0.5 on the host, so recip = 1/psum plain.
        if q not in recips:
            recips[q] = recip_pool.tile([128, WIN], BF16, tag="recip", name="recip")
        a, b = KR[k]
        if on_v:
            with nc.allow_low_precision("bf16 recip; tolerance 2e-2"):
                nc.vector.reciprocal(recips[q][:, a:b], adps[q][k][:, 0 : b - a])
        else:
            _act_recip(nc.scalar, recips[q][:, a:b], adps[q][k][:, 0 : b - a], 0.0, 1.0)

    def emit_mods(q, k, ebf, etr=None):
        if q not in modbfs:
            modbfs[q] = modbf_pool.tile([128, WIN], BF16, tag="modbf", name="modbf")
            modtrs[q] = modbf_pool.tile([128, WIN], BF16, tag="modtr", name="modtr")
        a, b = KR[k]
        ebf.tensor_tensor(
            modbfs[q][:, a:b], adjq[q][:, 128 * q + a : 128 * q + b],
            recips[q][:, a:b], mybir.AluOpType.mult,
        )
        (etr or ebf).tensor_tensor(
            modtrs[q][:, a:b], adjTq[q][:, 128 * q + a : 128 * q + b],
            recips[q][:, a:b], mybir.AluOpType.mult,
        )

    # Region m of ft accumulates BOTH the direct contribution (rows of
    # q-block m, for m<=1) and the transposed contributions.  Writer counts
    # per region drive start/stop flags.
    ftps = {}
    ft_left = [20]  # total ft matmuls: 6+7+2+2+2+1

    def _ft_mm(m, lhsT, rhs):
        # start=True clears has_written for the WHOLE bank, so only the
        # very first ft matmul may set it; per-element has_written then
        # handles first-write-vs-accumulate for each region.
        ft = ftps["t"]
        start = ft_left[0] == 20
        ft_left[0] -= 1
        nc.tensor.matmul(
            ft[:, FD * m : FD * m + FD], lhsT, rhs,
            start=start, stop=(ft_left[0] == 0),
            skip_group_check=True,
        )

    def emit_trfin(q, cs):
        """Direct chunks: transpose modbf chunk c + direct/transposed MMs."""
        if "t" not in ftps:
            ftps["t"] = finT_ps.tile([128, NCHT * FD], F32, tag="ft", name="ftps")
        for c in cs:
            tr = trfin_ps.tile([128, 128], BF16, tag="trfin", name="tr")
            nc.tensor.transpose(
                tr[:], modbfs[q][:, 128 * c : 128 * c + 128], identb[:]
            )
            modt = modt_pool.tile([128, 128], BF16, tag="mt", name="mt")
            nc.vector.tensor_copy(modt[:], tr[:])
            # direct: region q += modtT-chunk @ xwb1 chunk (m = q+c)
            _ft_mm(q, modt[:], xwb1[:, FD * (q + c) : FD * (q + c) + FD])
            # transposed: region m=q+c += modTr chunk c (contract i) @ xwr1[q]
            _ft_mm(
                q + c,
                modtrs[q][:, 128 * c : 128 * c + 128],
                xwr1[:, FD * q : FD * q + FD],
            )

    # pre_m = ft_m[0:F] - deg_m * xw_chunk_m  (+ xw_chunk_m + bias for the
    # core's own rows, m in {0,1}); host just SUMS the pre blocks.
    pre_sb = const.tile([128, NCHT * F], F32)
    negxwb = const.tile([128, NCHT * F], BF16)
    degs = const.tile([128, NCHT], F32)

    def emit_pm(ms):
        ft = ftps["t"]
        if ms[0] == 0:
            nc.vector.tensor_scalar(
                negxwb[:].rearrange("p (m f) -> p m f", f=F),
                xwb1[:].rearrange("p (m f) -> p m f", f=FD)[:, :, 0:F],
                -1.0, None, mybir.AluOpType.mult,
            )
        for m in ms:
            nc.vector.tensor_copy(
                degs[:, m : m + 1], ft[:, FD * m + F : FD * m + F + 1]
            )
            corr = const.tile([128, F], F32, tag="pcorr", name="pcorr")
            nc.vector.tensor_scalar(
                corr[:], negxwb[:, F * m : F * m + F], degs[:, m : m + 1], None,
                mybir.AluOpType.mult,
            )
            nc.vector.tensor_tensor(
                pre_sb[:, F * m : F * m + F], ft[:, FD * m : FD * m + F],
                corr[:], mybir.AluOpType.add,
            )
            if m <= 1:
                nc.vector.tensor_tensor(
                    pre_sb[:, F * m : F * m + F], pre_sb[:, F * m : F * m + F],
                    xwb_pre[:, F * m : F * m + F], mybir.AluOpType.add,
                )
        if ms[-1] == 4:
            nc.scalar.dma_start(
                io["out_pre"][256 : 128 * 5, :].rearrange("(m p) f -> p m f", p=128),
                pre_sb[:, 2 * F : 5 * F].rearrange("p (m f) -> p m f", f=F),
            )
        if ms[-1] == 5:
            nc.sync.dma_start(
                io["out_pre"][0:256, :].rearrange("(m p) f -> p m f", p=128),
                pre_sb[:, 0 : 2 * F].rearrange("p (m f) -> p m f", f=F),
            )
            nc.sync.dma_start(
                io["out_pre"][128 * 5 : 128 * 6, :], pre_sb[:, 5 * F : 6 * F]
            )

    NW = ND8 + NVB
    SPLIT = NW - KSPLIT
    prefix = orders[1][0 : PE_PRE1 + PE_PRE2]
    n_vb_pre = min(max(V_PRE, sum(1 for s in prefix if s["kind"] == "vb") + 4), NVB)
    n_a8_pre = min(sum(1 for s in prefix if s["kind"] == "a8") + 2, ND8)

    # ---- q0 ----
    emit_v_producers(0, set(range(NVB)))
    emit_a_producers(0, {0, 1, 2})
    emit_pe_weave(0, 0, 6)
    emit_s1s2_mms()
    emit_s1s2_copies()
    emit_a_producers(0, {3})
    emit_pe_weave(0, 6, 12)
    emit_xw_mms()
    emit_xw_copies_scalar()
    emit_a_producers(0, set(range(4, ND8)))
    emit_xw_copies_v()
    emit_pe_weave(0, 12, SPLIT)
    emit_pe_weave(0, SPLIT, NW, ks=(0,))
    emit_pe_folds(0, 0)
    emit_recip(0, 0)
    emit_mods(0, 0, nc.gpsimd, nc.vector)
    emit_pe_weave(0, SPLIT, NW, ks=(1,))
    emit_pe_folds(0, 1)

    # ---- q0 epilogue interleaved with q1 ----
    emit_v_producers(1, set(range(n_vb_pre)))
    emit_a_producers(1, set(range(1)))
    emit_recip(0, 1)
    emit_mods(0, 1, nc.vector)
    emit_a_producers(1, set(range(1, n_a8_pre)))
    emit_pe_weave(1, 0, PE_PRE1)
    emit_trfin(0, [0, 1, 2, 3])
    emit_pe_weave(1, PE_PRE1, PE_PRE1 + PE_PRE2)
    emit_trfin(0, [4])
    emit_pm([0])
    emit_v_producers(1, set(range(n_vb_pre, NVB)))
    emit_a_producers(1, set(range(n_a8_pre, ND8)))
    emit_pe_weave(1, PE_PRE1 + PE_PRE2, SPLIT)
    emit_pe_weave(1, SPLIT, NW, ks=(0,))
    emit_pe_folds(1, 0)
    emit_recip(1, 0)
    emit_mods(1, 0, nc.gpsimd, nc.vector)
    emit_pe_weave(1, SPLIT, SPLIT + 8, ks=(1,))
    emit_trfin(1, [0, 1])
    emit_pm([2])
    emit_pe_weave(1, SPLIT + 8, SPLIT + 16, ks=(1,))
    emit_trfin(1, [2, 3])
    emit_pm([3, 4])
    emit_pe_weave(1, SPLIT + 16, NW, ks=(1,))
    emit_pe_folds(1, 1)
    emit_recip(1, 1, on_v=True)
    emit_mods(1, 1, nc.vector)
    emit_trfin(1, [4])
    emit_pm([1, 5])
    _ = 0
    if "dbg_pm" in io:
        nc.sync.dma_start(io["dbg_pm"][:], pmall[:])
        for q in range(2):
            nc.sync.dma_start(io["dbg_mod"][128 * q : 128 * q + 128, :], modbfs[q][:])
            nc.sync.dma_start(io["dbg_tr"][128 * q : 128 * q + 128, :], modtrs[q][:])


_CACHE = {}


def _build(debug=False):
    if ("nc", debug) in _CACHE:
        return _CACHE[("nc", debug)]
    nc = bacc.Bacc()
    io = {
        "xallb": nc.declare_dram_parameter("xallb", [C, XALL2], BF16, isOutput=False),
        "adjb": nc.declare_dram_parameter("adjb", [R, W], BF16, isOutput=False),
        "adjTb": nc.declare_dram_parameter("adjTb", [R, W], BF16, isOutput=False),
        "out_pre": nc.declare_dram_parameter("out_pre", [NCHT * C, F], F32, isOutput=True),
    }
    if debug:
        io["dbg_pm"] = nc.declare_dram_parameter("dbg_pm", [C, NCHT * FD], F32, isOutput=True)
        io["dbg_mod"] = nc.declare_dram_parameter("dbg_mod", [R, WIN], BF16, isOutput=True)
        io["dbg_tr"] = nc.declare_dram_parameter("dbg_tr", [R, WIN], BF16, isOutput=True)
    with tile.TileContext(nc) as tc:
        _body(tc, io)
    nc.finalize()
    _CACHE[("nc", debug)] = nc
    return nc


def _make_in_maps(x, adj, weight, bias):
    in_maps = []
    ar = np.arange(N)
    for core in range(8):
        b, blk = core // 4, core % 4
        r0 = blk * R
        cols = (r0 + np.arange(W)) % N
        i = (r0 + np.arange(R))[:, None]
        j = ar[None, :]
        dd = (j - i) % N
        own = ((dd >= 1) & (dd <= 511)) | ((dd == 512) & (i < 512))
        own_w = own[:, cols]

        xallb = np.zeros((C, XALL2), dtype=ml_dtypes.bfloat16)
        xT = x[b].T.astype(ml_dtypes.bfloat16)
        xallb[:, 0:W] = xT[:, cols]
        xallb[:, W : W + R] = x[b, r0 : r0 + R].T.astype(ml_dtypes.bfloat16)
        xallb[:, W + R : W + R + F] = weight.astype(ml_dtypes.bfloat16)
        xallb[0, W + R + F : W + R + 2 * F] = bias.astype(ml_dtypes.bfloat16)

        adjb = (0.5 * adj[b, r0 : r0 + R][:, cols] * own_w).astype(ml_dtypes.bfloat16)
        adjTb = (0.5 * adj[b][:, r0 : r0 + R].T[:, cols] * own_w).astype(
            ml_dtypes.bfloat16
        )

        in_maps.append({"xallb": xallb, "adjb": adjb, "adjTb": adjTb})
    return in_maps


def run(x, adj, weight, bias, trace=False, debug=False):
    nc = _build(debug=debug)
    res = run_bass_kernel_spmd(
        nc, _make_in_maps(x, adj, weight, bias), list(range(8)), trace=trace
    )
    out = np.zeros((B, N, F), dtype=np.float32)
    for core in range(8):
        b, blk = core // 4, core % 4
        r0 = blk * R
        pre = res.results[core]["out_pre"]  # [6*128, F], rotated chunks
        for m in range(NCHT):
            rows = (r0 + 128 * m) % N
            out[b, rows : rows + 128] += pre[128 * m : 128 * m + 128]
    return out, res


def kernel(x, adj, weight, bias):
    x = np.asarray(x, dtype=np.float32)
    adj = np.asarray(adj, dtype=np.float32)
    weight = np.asarray(weight, dtype=np.float32)
    bias = np.asarray(bias, dtype=np.float32)
    out, _ = run(x, adj, weight, bias, trace=False)
    return out


# revision 17
# speedup vs baseline: 1.1758x; 1.1758x over previous
"""DenseGTVConv Trainium2 kernel (v8, band-symmetric + ReduceScatter).

out = (I - (D - A~)) @ (x @ W) + bias,  A~ = adj / clamp(pairwise_L1(xW), 1e-3)

D is symmetric: each unordered pair {i,j} is computed ONCE.  Core c (of 4
per batch) owns rows [256c, 256c+256) and the pairs (i, j) with
(j-i) mod 1024 in [1, 511] (plus the d==512 tie when i < 512).  Inputs are
rotated by r0 so each core sees a contiguous 768-column band; adj and adjT
slices are host-masked to the owned band, so unwanted D entries multiply
to zero.  Per owned pair the core computes BOTH mod[i,j] (direct, via adj)
and mod[j,i] (transposed, via adjT); direct contributions use the usual
transpose+matmul, transposed contributions contract naturally over i
(no transpose).  Per-core partials [1024, 65] (out | deg) are scattered to
absolute rows via indirect DMA (host-provided indices), ReduceScattered
across the 4-core group, and the final combine runs on-chip.

Hot-loop structure (relu-identity + sliding-E PSUM reduction, fp8
DoubleRow duos, S1/S2 folds, software pipelining) follows v5-v7, with a
640-column window per 128-row block (PSUM split 512 + 128).

Self-contained: hardcoded shapes for B=2, N=1024, F_in=128, F_out=64.
"""
import sys

sys.path.insert(0, "/opt/trn_rl_repo")

from contextlib import ExitStack

import numpy as np
import ml_dtypes

import concourse.bass as bass
import concourse.bacc as bacc
import concourse.tile as tile
from concourse.masks import make_identity
from concourse import mybir
from concourse._compat import with_exitstack
from concourse.bass_utils import run_bass_kernel_spmd

F32 = mybir.dt.float32
BF16 = mybir.dt.bfloat16
FP8 = mybir.dt.float8e4
U32 = mybir.dt.uint32

B, N, C, F = 2, 1024, 128, 64
R = 256
NPAIR = R // 2
EPS = 1e-3

W = 768  # band columns per core (rotated)
WIN = 640  # window per 128-row q-block: jj in [128q, 128q+640)
K0, K1 = 512, 128  # PSUM column split of the window
NCHD = 5  # direct 128-col chunks per q (640/128)
NCHT = 6  # total jj chunks per core (768/128)
FD = F + 1  # 65: out | deg

# xallb layout [128, W + R + 2F = 1152]:
#   0:768 xT' (rotated), 768:1024 xrT, 1024:1088 W, 1088:1152 bias(row0)
XALL2 = W + R + 2 * F

ND8 = 10  # fp8 duos per q (pairs d, d+32) on ACT
NVB = 64 - 2 * ND8  # bf16 vector pairs per q

NWARM = 4
KSPLIT = 24
V_PRE = 14
PE_PRE1 = 6
PE_PRE2 = 7


def _expand_sched():
    slots = [dict(kind="a8", d=d) for d in range(ND8)]
    used = set(range(ND8)) | set(range(32, 32 + ND8))
    slots += [dict(kind="vb", t=t) for t in range(64) if t not in used]
    return slots


def _pe_order(slots):
    vb = [s for s in slots if s["kind"] == "vb"]
    a8 = [s for s in slots if s["kind"] == "a8"]
    if not a8 or not vb:
        return slots
    keyed = [((i + 0.5) / len(vb), s) for i, s in enumerate(vb)]
    keyed += [((j + 1.5) / (len(a8) + 1), s) for j, s in enumerate(a8)]
    return [s for _, s in sorted(keyed, key=lambda p: p[0])]


def _act_recip(sc, out, in_, bias, scale=1.0):
    inputs = [sc.lower_ap(in_)]
    for arg in (bias, scale, 0.0):
        inputs.append(mybir.ImmediateValue(dtype=mybir.dt.float32, value=arg))
    return sc.add_instruction(
        mybir.InstActivation(
            name=sc.bass.get_next_instruction_name(),
            func=mybir.ActivationFunctionType.Reciprocal,
            ins=inputs,
            outs=[sc.lower_ap(out)],
        )
    )


@with_exitstack
def _body(ctx: ExitStack, tc: "tile.TileContext", io: dict):
    nc = tc.nc
    const = ctx.enter_context(tc.tile_pool(name="const", bufs=1))
    tmpv_pool = ctx.enter_context(tc.tile_pool(name="tmpv", bufs=14))
    tmp8a_pool = ctx.enter_context(tc.tile_pool(name="tmp8a", bufs=5))
    recip_pool = ctx.enter_context(tc.tile_pool(name="recip", bufs=2))
    modbf_pool = ctx.enter_context(tc.tile_pool(name="modbf", bufs=2))
    modt_pool = ctx.enter_context(tc.tile_pool(name="modt", bufs=3))
    setup_ps = ctx.enter_context(tc.tile_pool(name="sps", bufs=2, space="PSUM"))
    ad_ps = ctx.enter_context(tc.tile_pool(name="adps", bufs=2, space="PSUM"))
    trfin_ps = ctx.enter_context(tc.tile_pool(name="trfin", bufs=2, space="PSUM"))
    finT_ps = ctx.enter_context(tc.tile_pool(name="ftps", bufs=1, space="PSUM"))

    xallb = const.tile([128, XALL2], BF16)
    adjq = [const.tile([128, W], BF16, tag=f"adj{q}", name=f"adj{q}") for q in range(2)]
    adjTq = [
        const.tile([128, W], BF16, tag=f"adjT{q}", name=f"adjT{q}") for q in range(2)
    ]
    junk = const.tile([128, 512], BF16)
    nc.vector.memset(junk[:], 0.0)

    # ---- input DMAs ----
    nc.sync.dma_start(xallb[:, W:XALL2], io["xallb"][:, W:XALL2])
    nc.sync.dma_start(xallb[:, 0:256], io["xallb"][:, 0:256])
    nc.scalar.dma_start(xallb[:, 256:W], io["xallb"][:, 256:W])
    nc.scalar.dma_start(adjq[0][:], io["adjb"][0:128, :])
    nc.scalar.dma_start(adjq[1][:], io["adjb"][128:256, :])
    nc.gpsimd.dma_start(adjTq[0][:], io["adjTb"][0:128, :])
    nc.gpsimd.dma_start(adjTq[1][:], io["adjTb"][128:256, :])

    # ---- PE warmup ----
    for w in range(NWARM):
        ps = ad_ps.tile([128, 512], F32, tag="adps0", name=f"warm{w}")
        nc.tensor.matmul(ps[:], junk[:, 0:128], junk[:], start=True, stop=True)

    # ---- constants ----
    onesrow = const.tile([1, 512], BF16)
    nc.vector.memset(onesrow[:], 1.0)
    neghalf = const.tile([1, 128], BF16)
    nc.vector.memset(neghalf[:], -0.5)
    ones64b = const.tile([64, 1], BF16)
    nc.vector.memset(ones64b[:], 1.0)
    ones64f = const.tile([64, 1], F32)
    nc.vector.memset(ones64f[:], 1.0)
    s1row = const.tile([1, W], BF16)
    s2row = const.tile([1, R], BF16)

    identb = const.tile([128, 128], BF16)
    make_identity(nc, identb[:])
    Eb = const.tile([128, 254], BF16)
    nc.gpsimd.memset(Eb[:], 0.0)
    nc.gpsimd.memset(Eb[0:64, 126:127], 1.0)
    nc.gpsimd.memset(Eb[64:128, 127:128], 1.0)
    Eb8 = const.tile([128, 320], FP8)
    nc.gpsimd.memset(Eb8[:], 0.0)
    nc.gpsimd.memset(Eb8[0:64, 62:63], 1.0)
    nc.gpsimd.memset(Eb8[64:128, 63:64], 1.0)
    nc.gpsimd.memset(Eb8[0:64, 254:255], 1.0)
    nc.gpsimd.memset(Eb8[64:128, 255:256], 1.0)

    xTb = xallb[:, 0:W]
    xrTb = xallb[:, W : W + R]
    w_sb = xallb[:, W + R : W + R + F]
    bias_sb = xallb[0:1, W + R + F : W + R + 2 * F]

    # ---- dbl' [128, 768] ----
    w2 = const.tile([128, 128], BF16)
    nc.vector.tensor_copy(w2[:, 0:F], w_sb)
    nc.vector.tensor_copy(w2[:, F : 2 * F], w_sb)
    dbl = const.tile([128, W], BF16)
    ps = ad_ps.tile([128, 512], F32, tag="adps0", name="dblps")
    nc.tensor.matmul(ps[:], w2[:], xTb[:, 0:512], start=True, stop=True)
    nc.vector.tensor_copy(dbl[:, 0:512], ps[:])
    ps = ad_ps.tile([128, 512], F32, tag="adps0", name="dblps")
    nc.tensor.matmul(ps[:, 0:256], w2[:], xTb[:, 512:W], start=True, stop=True)
    nc.vector.tensor_copy(dbl[:, 512:W], ps[:, 0:256])

    # ---- xwT_rows / S / negS (exact i-side) ----
    xwT_rows = const.tile([64, R], F32)
    ps = ad_ps.tile([128, 512], F32, tag="adps0", name="xwTps")
    nc.tensor.matmul(ps[0:64, 0:R], w_sb, xrTb[:], start=True, stop=True)
    nc.vector.tensor_copy(xwT_rows[:], ps[0:64, 0:R])
    S_bf = const.tile([128, NPAIR], F32)
    nc.vector.tensor_copy(S_bf[0:64, :], xwT_rows[:, 0:R:2])
    nc.vector.tensor_copy(S_bf[64:128, :], xwT_rows[:, 1:R:2])
    negS = const.tile([128, NPAIR], F32)
    nc.vector.tensor_scalar(negS[:], S_bf[:], -1.0, None, mybir.AluOpType.mult)

    # ---- deferred setup: s1/s2 ----
    s1ps = []

    def emit_s1s2_mms():
        ps = setup_ps.tile([128, 512], F32, tag="sps", name="s1psA")
        nc.tensor.matmul(ps[0:1, :], ones64b[:], dbl[0:64, 0:512], start=True, stop=True)
        s1ps.append(ps)
        ps = setup_ps.tile([128, 512], F32, tag="sps", name="s1psB")
        nc.tensor.matmul(
            ps[0:1, 0:256], ones64b[:], dbl[0:64, 512:W], start=True, stop=True
        )
        s1ps.append(ps)
        ps = setup_ps.tile([128, 512], F32, tag="sps", name="s2ps")
        nc.tensor.matmul(ps[0:1, 0:R], ones64f[:], xwT_rows[:], start=True, stop=True)
        s1ps.append(ps)

    def emit_s1s2_copies():
        nc.scalar.copy(s1row[:, 0:512], s1ps[0][0:1, :])
        nc.scalar.copy(s1row[:, 512:W], s1ps[1][0:1, 0:256])
        nc.scalar.activation(
            s2row[:], s1ps[2][0:1, 0:R],
            mybir.ActivationFunctionType.Copy, bias=0.5 * EPS, scale=0.5,
        )

    # ---- deferred setup: xwb1 (6 chunks), xw_rows, bias, xwr1 ----
    xwb1 = const.tile([128, NCHT * FD], BF16)
    xw_rows = const.tile([128, 2 * F], F32)
    negxw = const.tile([128, 2 * F], F32)
    xwb_pre = const.tile([128, 2 * F], F32)
    xwr1 = const.tile([128, 2 * FD], BF16)
    ones1 = const.tile([1, 128], BF16)
    nc.scalar.activation(
        ones1[:], xallb[0:1, 0:128], mybir.ActivationFunctionType.Copy,
        bias=1.0, scale=0.0,
    )
    xwps = []

    def emit_xw_mms():
        ps_xw = setup_ps.tile([128, 512], F32, tag="sps", name="psxw")
        for c in range(NCHT):
            nc.tensor.matmul(
                ps_xw[:, F * c : F * c + F], xTb[:, 128 * c : 128 * c + 128], w_sb,
                start=True, stop=True,
            )
        ps2 = setup_ps.tile([128, 512], F32, tag="sps", name="psxr")
        for q in range(2):
            nc.tensor.matmul(
                ps2[:, F * q : F * q + F], xrTb[:, 128 * q : 128 * q + 128], w_sb,
                start=True, stop=True,
            )
            nc.tensor.matmul(
                ps2[:, 2 * F + F * q : 3 * F + F * q], ones1[:], bias_sb,
                start=True, stop=True,
            )
        xwps.extend([ps_xw, ps2])

    def emit_xw_copies_scalar():
        nc.scalar.copy(
            xwb1[:].rearrange("p (c f) -> p c f", c=NCHT)[:, :, 0:F],
            xwps[0][:, 0 : NCHT * F].rearrange("p (c f) -> p c f", c=NCHT),
        )

    def emit_xw_copies_v():
        nc.vector.memset(xwb1[:, F : NCHT * FD : FD], 1.0)
        nc.vector.tensor_copy(xw_rows[:], xwps[1][:, 0 : 2 * F])
        nc.vector.tensor_scalar(negxw[:], xw_rows[:], -1.0, None, mybir.AluOpType.mult)
        nc.vector.tensor_tensor(
            xwb_pre[:], xw_rows[:], xwps[1][:, 2 * F : 4 * F], mybir.AluOpType.add
        )
        for q in range(2):
            nc.vector.tensor_copy(
                xwr1[:, FD * q : FD * q + F], xw_rows[:, F * q : F * q + F]
            )
        nc.vector.memset(xwr1[:, F : 2 * FD : FD], 1.0)

    out_sb = [const.tile([128, F], F32, tag=f"osb{q}", name=f"osb{q}") for q in range(2)]

    # ================= hot loop =================
    slots = [_expand_sched() for _ in range(2)]
    orders = [_pe_order(slots[q]) for q in range(2)]
    adps = {}
    for q in range(2):
        adps[q] = [
            ad_ps.tile(
                [128, 512] if k == 0 else [128, 128], F32,
                tag=f"adps{k}", name=f"adps{q}_{k}",
                bufs=(2 if k == 0 else 1),
            )
            for k in range(2)
        ]

    def emit_v_producers(q, idxs):
        for j, s in enumerate(x for x in slots[q] if x["kind"] == "vb"):
            if j not in idxs:
                continue
            t = 64 * q + s["t"]
            tmpb = tmpv_pool.tile([128, WIN], BF16, tag="tv", name="tv")
            nc.vector.tensor_scalar(
                tmpb[:], dbl[:, 128 * q : 128 * q + WIN], S_bf[:, t : t + 1], 0.0,
                mybir.AluOpType.subtract, mybir.AluOpType.max,
            )
            s["tile"] = tmpb

    def emit_a_producers(q, idxs):
        for j, s in enumerate(x for x in slots[q] if x["kind"] == "a8"):
            if j not in idxs:
                continue
            d = s["d"]
            duo = tmp8a_pool.tile([128, 2 * WIN], FP8, tag="ta", name="ta")
            for half, t in ((0, 64 * q + d), (1, 64 * q + d + 32)):
                nc.scalar.activation(
                    duo[:, WIN * half : WIN * half + WIN],
                    dbl[:, 128 * q : 128 * q + WIN],
                    mybir.ActivationFunctionType.Relu,
                    bias=negS[:, t : t + 1], scale=1.0,
                )
            s["tile"] = duo

    KR = {0: (0, K0), 1: (K0, WIN)}  # window column ranges per psum half

    def emit_pe_weave(q, lo, hi, ks=(0, 1)):
        order = orders[q]
        for i in range(lo, hi):
            s = order[i]
            for k in ks:
                a, b = KR[k]
                if s["kind"] == "vb":
                    r = s["t"]
                    nc.tensor.matmul(
                        adps[q][k][:, 0 : b - a],
                        Eb[:, 126 - 2 * r : 254 - 2 * r],
                        s["tile"][:, a:b],
                        start=(i == 0),
                        stop=False,
                    )
                else:
                    d = s["d"]
                    lhsT = Eb8[:, 62 - 2 * d : 62 - 2 * d + 256].rearrange(
                        "p (s m) -> p s m", s=2
                    )
                    rhs = s["tile"][:].rearrange("p (s n) -> p s n", s=2)[:, :, a:b]
                    nc.tensor.matmul(
                        adps[q][k][:, 0 : b - a], lhsT, rhs,
                        start=(i == 0), stop=False,
                        perf_mode=mybir.MatmulPerfMode.DoubleRow,
                    )

    def emit_pe_folds(q, k):
        a, b = KR[k]
        nc.tensor.matmul(
            adps[q][k][:, 0 : b - a], neghalf[:],
            s1row[0:1, 128 * q + a : 128 * q + b],
            start=False, stop=False,
        )
        nc.tensor.matmul(
            adps[q][k][:, 0 : b - a], s2row[0:1, 128 * q : 128 * q + 128],
            onesrow[0:1, 0 : b - a],
            start=False, stop=True,
        )

    recips = {}
    modbfs = {}
    modtrs = {}

    def emit_recip(q, k):
        if q not in recips:
            recips[q] = recip_pool.tile([128, WIN], BF16, tag="recip", name="recip")
        a, b = KR[k]
        _act_recip(nc.scalar, recips[q][:, a:b], adps[q][k][:, 0 : b - a], 0.0, 2.0)

    def emit_mods(q, k, ebf, etr=None):
        if q not in modbfs:
            modbfs[q] = modbf_pool.tile([128, WIN], BF16, tag="modbf", name="modbf")
            modtrs[q] = modbf_pool.tile([128, WIN], BF16, tag="modtr", name="modtr")
        a, b = KR[k]
        ebf.tensor_tensor(
            modbfs[q][:, a:b], adjq[q][:, 128 * q + a : 128 * q + b],
            recips[q][:, a:b], mybir.AluOpType.mult,
        )
        (etr or ebf).tensor_tensor(
            modtrs[q][:, a:b], adjTq[q][:, 128 * q + a : 128 * q + b],
            recips[q][:, a:b], mybir.AluOpType.mult,
        )

    # Region m of ft accumulates BOTH the direct contribution (rows of
    # q-block m, for m<=1) and the transposed contributions.  Writer counts
    # per region drive start/stop flags.
    ftps = {}
    ft_left = [20]  # total ft matmuls: 6+7+2+2+2+1

    def _ft_mm(m, lhsT, rhs):
        # start=True clears has_written for the WHOLE bank, so only the
        # very first ft matmul may set it; per-element has_written then
        # handles first-write-vs-accumulate for each region.
        ft = ftps["t"]
        start = ft_left[0] == 20
        ft_left[0] -= 1
        nc.tensor.matmul(
            ft[:, FD * m : FD * m + FD], lhsT, rhs,
            start=start, stop=(ft_left[0] == 0),
            skip_group_check=True,
        )

    def emit_trfin(q, cs):
        """Direct chunks: transpose modbf chunk c + direct/transposed MMs."""
        if "t" not in ftps:
            ftps["t"] = finT_ps.tile([128, NCHT * FD], F32, tag="ft", name="ftps")
        for c in cs:
            tr = trfin_ps.tile([128, 128], BF16, tag="trfin", name="tr")
            nc.tensor.transpose(
                tr[:], modbfs[q][:, 128 * c : 128 * c + 128], identb[:]
            )
            modt = modt_pool.tile([128, 128], BF16, tag="mt", name="mt")
            nc.vector.tensor_copy(modt[:], tr[:])
            # direct: region q += modtT-chunk @ xwb1 chunk (m = q+c)
            _ft_mm(q, modt[:], xwb1[:, FD * (q + c) : FD * (q + c) + FD])
            # transposed: region m=q+c += modTr chunk c (contract i) @ xwr1[q]
            _ft_mm(
                q + c,
                modtrs[q][:, 128 * c : 128 * c + 128],
                xwr1[:, FD * q : FD * q + FD],
            )

    # pre_m = ft_m[0:F] - deg_m * xw_chunk_m  (+ xw_chunk_m + bias for the
    # core's own rows, m in {0,1}); host just SUMS the pre blocks.
    pre_sb = const.tile([128, NCHT * F], F32)
    negxwb = const.tile([128, NCHT * F], BF16)
    degs = const.tile([128, NCHT], F32)

    def emit_pm(ms):
        ft = ftps["t"]
        if ms[0] == 0:
            nc.vector.tensor_scalar(
                negxwb[:].rearrange("p (m f) -> p m f", f=F),
                xwb1[:].rearrange("p (m f) -> p m f", f=FD)[:, :, 0:F],
                -1.0, None, mybir.AluOpType.mult,
            )
        for m in ms:
            nc.vector.tensor_copy(
                degs[:, m : m + 1], ft[:, FD * m + F : FD * m + F + 1]
            )
            corr = const.tile([128, F], F32, tag="pcorr", name="pcorr")
            nc.vector.tensor_scalar(
                corr[:], negxwb[:, F * m : F * m + F], degs[:, m : m + 1], None,
                mybir.AluOpType.mult,
            )
            nc.vector.tensor_tensor(
                pre_sb[:, F * m : F * m + F], ft[:, FD * m : FD * m + F],
                corr[:], mybir.AluOpType.add,
            )
            if m <= 1:
                nc.vector.tensor_tensor(
                    pre_sb[:, F * m : F * m + F], pre_sb[:, F * m : F * m + F],
                    xwb_pre[:, F * m : F * m + F], mybir.AluOpType.add,
                )
        if ms[-1] == 4:
            nc.scalar.dma_start(
                io["out_pre"][256 : 128 * 5, :].rearrange("(m p) f -> p m f", p=128),
                pre_sb[:, 2 * F : 5 * F].rearrange("p (m f) -> p m f", f=F),
            )
        if ms[-1] == 5:
            nc.sync.dma_start(
                io["out_pre"][0:256, :].rearrange("(m p) f -> p m f", p=128),
                pre_sb[:, 0 : 2 * F].rearrange("p (m f) -> p m f", f=F),
            )
            nc.sync.dma_start(
                io["out_pre"][128 * 5 : 128 * 6, :], pre_sb[:, 5 * F : 6 * F]
            )

    NW = ND8 + NVB
    SPLIT = NW - KSPLIT
    prefix = orders[1][0 : PE_PRE1 + PE_PRE2]
    n_vb_pre = min(max(V_PRE, sum(1 for s in prefix if s["kind"] == "vb") + 4), NVB)
    n_a8_pre = min(sum(1 for s in prefix if s["kind"] == "a8") + 2, ND8)

    # ---- q0 ----
    emit_v_producers(0, set(range(NVB)))
    emit_a_producers(0, {0, 1, 2})
    emit_pe_weave(0, 0, 6)
    emit_s1s2_mms()
    emit_s1s2_copies()
    emit_a_producers(0, {3})
    emit_pe_weave(0, 6, 12)
    emit_xw_mms()
    emit_xw_copies_scalar()
    emit_a_producers(0, set(range(4, ND8)))
    emit_xw_copies_v()
    emit_pe_weave(0, 12, SPLIT)
    emit_pe_weave(0, SPLIT, NW, ks=(0,))
    emit_pe_folds(0, 0)
    emit_recip(0, 0)
    emit_mods(0, 0, nc.gpsimd, nc.vector)
    emit_pe_weave(0, SPLIT, NW, ks=(1,))
    emit_pe_folds(0, 1)

    # ---- q0 epilogue interleaved with q1 ----
    emit_v_producers(1, set(range(n_vb_pre)))
    emit_a_producers(1, set(range(1)))
    emit_recip(0, 1)
    emit_mods(0, 1, nc.vector)
    emit_a_producers(1, set(range(1, n_a8_pre)))
    emit_pe_weave(1, 0, PE_PRE1)
    emit_trfin(0, [0, 1, 2, 3])
    emit_pe_weave(1, PE_PRE1, PE_PRE1 + PE_PRE2)
    emit_trfin(0, [4])
    emit_pm([0])
    emit_v_producers(1, set(range(n_vb_pre, NVB)))
    emit_a_producers(1, set(range(n_a8_pre, ND8)))
    emit_pe_weave(1, PE_PRE1 + PE_PRE2, SPLIT)
    emit_pe_weave(1, SPLIT, NW, ks=(0,))
    emit_pe_folds(1, 0)
    emit_recip(1, 0)
    emit_mods(1, 0, nc.gpsimd, nc.vector)
    emit_pe_weave(1, SPLIT, SPLIT + 8, ks=(1,))
    emit_trfin(1, [0, 1])
    emit_pm([2])
    emit_pe_weave(1, SPLIT + 8, SPLIT + 16, ks=(1,))
    emit_trfin(1, [2, 3])
    emit_pm([3, 4])
    emit_pe_weave(1, SPLIT + 16, NW, ks=(1,))
    emit_pe_folds(1, 1)
    emit_recip(1, 1)
    emit_mods(1, 1, nc.vector)
    emit_trfin(1, [4])
    emit_pm([1, 5])
    _ = 0
    if "dbg_pm" in io:
        nc.sync.dma_start(io["dbg_pm"][:], pmall[:])
        for q in range(2):
            nc.sync.dma_start(io["dbg_mod"][128 * q : 128 * q + 128, :], modbfs[q][:])
            nc.sync.dma_start(io["dbg_tr"][128 * q : 128 * q + 128, :], modtrs[q][:])


_CACHE = {}


def _build(debug=False):
    if ("nc", debug) in _CACHE:
        return _CACHE[("nc", debug)]
    nc = bacc.Bacc()
    io = {
        "xallb": nc.declare_dram_parameter("xallb", [C, XALL2], BF16, isOutput=False),
        "adjb": nc.declare_dram_parameter("adjb", [R, W], BF16, isOutput=False),
        "adjTb": nc.declare_dram_parameter("adjTb", [R, W], BF16, isOutput=False),
        "out_pre": nc.declare_dram_parameter("out_pre", [NCHT * C, F], F32, isOutput=True),
    }
    if debug:
        io["dbg_pm"] = nc.declare_dram_parameter("dbg_pm", [C, NCHT * FD], F32, isOutput=True)
        io["dbg_mod"] = nc.declare_dram_parameter("dbg_mod", [R, WIN], BF16, isOutput=True)
        io["dbg_tr"] = nc.declare_dram_parameter("dbg_tr", [R, WIN], BF16, isOutput=True)
    with tile.TileContext(nc) as tc:
        _body(tc, io)
    nc.finalize()
    _CACHE[("nc", debug)] = nc
    return nc


def _make_in_maps(x, adj, weight, bias):
    in_maps = []
    ar = np.arange(N)
    for core in range(8):
        b, blk = core // 4, core % 4
        r0 = blk * R
        cols = (r0 + np.arange(W)) % N
        i = (r0 + np.arange(R))[:, None]
        j = ar[None, :]
        dd = (j - i) % N
        own = ((dd >= 1) & (dd <= 511)) | ((dd == 512) & (i < 512))
        own_w = own[:, cols]

        xallb = np.zeros((C, XALL2), dtype=ml_dtypes.bfloat16)
        xT = x[b].T.astype(ml_dtypes.bfloat16)
        xallb[:, 0:W] = xT[:, cols]
        xallb[:, W : W + R] = x[b, r0 : r0 + R].T.astype(ml_dtypes.bfloat16)
        xallb[:, W + R : W + R + F] = weight.astype(ml_dtypes.bfloat16)
        xallb[0, W + R + F : W + R + 2 * F] = bias.astype(ml_dtypes.bfloat16)

        adjb = (adj[b, r0 : r0 + R][:, cols] * own_w).astype(ml_dtypes.bfloat16)
        adjTb = (adj[b][:, r0 : r0 + R].T[:, cols] * own_w).astype(ml_dtypes.bfloat16)

        in_maps.append({"xallb": xallb, "adjb": adjb, "adjTb": adjTb})
    return in_maps


def run(x, adj, weight, bias, trace=False, debug=False):
    nc = _build(debug=debug)
    res = run_bass_kernel_spmd(
        nc, _make_in_maps(x, adj, weight, bias), list(range(8)), trace=trace
    )
    out = np.zeros((B, N, F), dtype=np.float32)
    for core in range(8):
        b, blk = core // 4, core % 4
        r0 = blk * R
        pre = res.results[core]["out_pre"]  # [6*128, F], rotated chunks
        for m in range(NCHT):
            rows = (r0 + 128 * m) % N
            out[b, rows : rows + 128] += pre[128 * m : 128 * m + 128]
    return out, res


def kernel(x, adj, weight, bias):
    x = np.asarray(x, dtype=np.float32)
    adj = np.asarray(adj, dtype=np.float32)
    weight = np.asarray(weight, dtype=np.float32)
    bias = np.asarray(bias, dtype=np.float32)
    out, _ = run(x, adj, weight, bias, trace=False)
    return out
